# revision 9
# baseline (speedup 1.0000x reference)
"""GCMC all-pairs message-passing kernel for Trainium2 (Bass/Tile), 8 NeuronCores.

Math (reference semantics):
  pos = (s[:, :2] + s[:, 2:4])/2,  vel = (s[:, 4:6] + s[:, 6:8])/2
  dist[i,j] = |pos_i - pos_j|, alpha = softmax(-dist/tau, diag masked)
  mu_i  = q_i - (alpha @ q)_i            with q = [pos, vel]  (N,4)
  sig_i = sqrt((alpha @ q^2)_i - (alpha @ q)_i^2 + 1e-6)
  f_i = [mu, sig];  MLP: h = relu(f W1^T + b1); dv = h Wc^T + bc
  sigma2 = softplus(h Wu^T + bu) + 0.01

Sharding: rows i split across 8 cores (768 rows each), each core sees all j.
Per core, layout is [j-partition (128) x i-free (768)] per j-chunk; 48 chunks.
dist2 comes from one K=4 fp32 matmul (expanded form), sqrt/exp on ScalarE
(batched by ACT table set with a full-size SBUF staging buffer), the
alpha-weighted sums from a K=128 fp32 matmul accumulating [9 x 768] in PSUM.

Per-core j-chunks are rotated by host prep so that the 6 diagonal-overlap
chunks are always the first 6 the program processes (SPMD: one program, the
rotation lives in the per-core input data).
"""

import numpy as np

import concourse.bass as bass
import concourse.bacc as bacc
import concourse.tile as tile
import concourse.mybir as mybir
from concourse.bass_utils import run_bass_kernel_spmd

N = 6144
NCORES = 8
SLAB = N // NCORES            # 768 rows of i per core
NCH = N // 128                # 48 j-chunks
DIAG = SLAB // 128            # 6 chunks per core overlap the diagonal
F32 = mybir.dt.float32

_CACHE = {}


def _build(tau: float):
    inv_tau2 = float(1.0 / (tau * tau))

    nc = bacc.Bacc(
        "TRN2", target_bir_lowering=False, debug=False, num_devices=NCORES
    )

    Arep_d = nc.dram_tensor("Arep", [4, N], F32, kind="ExternalInput")
    Brep_d = nc.dram_tensor("Brep", [4, SLAB], F32, kind="ExternalInput")
    Q_d = nc.dram_tensor("Qrows", [N, 9], F32, kind="ExternalInput")
    mask_d = nc.dram_tensor("dmask", [128, 128], F32, kind="ExternalInput")
    qT_d = nc.dram_tensor("qT", [4, SLAB], F32, kind="ExternalInput")
    W1T_d = nc.dram_tensor("W1T", [8, 16], F32, kind="ExternalInput")
    b1_d = nc.dram_tensor("b1c", [16, 1], F32, kind="ExternalInput")
    WcT_d = nc.dram_tensor("WcT", [16, 2], F32, kind="ExternalInput")
    bc_d = nc.dram_tensor("bcc", [2, 1], F32, kind="ExternalInput")
    WuT_d = nc.dram_tensor("WuT", [16, 2], F32, kind="ExternalInput")
    bu_d = nc.dram_tensor("buc", [2, 1], F32, kind="ExternalInput")

    dv_out = nc.dram_tensor("dvT", [2, SLAB], F32, kind="ExternalOutput")
    s2_out = nc.dram_tensor("s2T", [2, SLAB], F32, kind="ExternalOutput")
    fi_out = nc.dram_tensor("fiT", [8, SLAB], F32, kind="ExternalOutput")

    AF = mybir.ActivationFunctionType
    OP = mybir.AluOpType

    with tile.TileContext(nc) as tc:
        with tc.tile_pool(name="consts", bufs=1) as consts:
            # ---- constants / small inputs
            brep = consts.tile([4, SLAB], F32)
            nc.sync.dma_start(out=brep, in_=Brep_d[:, :])
            qsb = consts.tile([128, NCH, 9], F32)
            nc.sync.dma_start(
                out=qsb, in_=Q_d.rearrange("(c p) d -> p c d", p=128)
            )
            dmask = consts.tile([128, 128], F32)
            nc.sync.dma_start(out=dmask, in_=mask_d[:, :])
            qslab = consts.tile([4, SLAB], F32)
            nc.sync.dma_start(out=qslab, in_=qT_d[:, :])
            w1t = consts.tile([8, 16], F32)
            nc.sync.dma_start(out=w1t, in_=W1T_d[:, :])
            b1c = consts.tile([16, 1], F32)
            nc.sync.dma_start(out=b1c, in_=b1_d[:, :])
            wct = consts.tile([16, 2], F32)
            nc.sync.dma_start(out=wct, in_=WcT_d[:, :])
            bcc = consts.tile([2, 1], F32)
            nc.sync.dma_start(out=bcc, in_=bc_d[:, :])
            wut = consts.tile([16, 2], F32)
            nc.sync.dma_start(out=wut, in_=WuT_d[:, :])
            buc = consts.tile([2, 1], F32)
            nc.sync.dma_start(out=buc, in_=bu_d[:, :])

            # survives the big-buffer scope: alpha-weighted sums + row sums
            s_sb = consts.tile([9, SLAB], F32)

            with (
                tc.tile_pool(name="stage", bufs=1) as stagep,
                tc.tile_pool(name="lhs", bufs=3) as lhsp,
                tc.tile_pool(name="d2c", bufs=2) as d2cp,
                tc.tile_pool(name="etile", bufs=3) as ep,
            ):
                # dist/tau staging for all 48 chunks (fp32, 144KB/partition)
                stage = stagep.tile([128, NCH, SLAB], F32)

                # ---- phase 1: dist2 (PE) -> clamp (DVE) -> dist/tau (ACT)
                with tc.tile_pool(name="d2ps", bufs=2, space="PSUM") as d2ps:
                    for g in range(NCH // 2):
                        d2 = d2ps.tile([128, 2, 1024], F32)
                        d2c = d2cp.tile([128, 2, SLAB], F32)
                        for h in (0, 1):
                            t = 2 * g + h
                            lhsA = lhsp.tile([4, 128], F32)
                            nc.sync.dma_start(
                                out=lhsA, in_=Arep_d[:, bass.ts(t, 128)]
                            )
                            nc.tensor.matmul(
                                d2[:, h, 0:512], lhsA, brep[:, 0:512],
                                start=True, stop=True,
                            )
                            nc.tensor.matmul(
                                d2[:, h, 512:SLAB], lhsA, brep[:, 512:SLAB],
                                start=True, stop=True,
                            )
                        # clamp fp32 cancellation negatives (else sqrt -> NaN)
                        nc.vector.tensor_scalar_max(d2c, d2[:, :, 0:SLAB], 0.0)
                        # dist/tau = sqrt(d2 / tau^2)
                        nc.scalar.activation(
                            stage[:, 2 * g : 2 * g + 2, :], d2c, AF.Sqrt,
                            scale=inv_tau2,
                        )

                # ---- phase 2: E = exp(-dist/tau), diag mask, S += Q^T E
                with tc.tile_pool(name="accps", bufs=1, space="PSUM") as accps:
                    acc_a = accps.tile([128, 512], F32)
                    acc_b = accps.tile([128, 256], F32)
                    for t in range(NCH):
                        e = ep.tile([128, SLAB], F32)
                        nc.scalar.activation(
                            e, stage[:, t, :], AF.Exp, scale=-1.0
                        )
                        if t < DIAG:
                            # chunks are host-rotated: diagonal block of
                            # chunk t sits at local columns [128t, 128t+128)
                            nc.vector.tensor_mul(
                                e[:, bass.ts(t, 128)],
                                e[:, bass.ts(t, 128)],
                                dmask,
                            )
                        nc.tensor.matmul(
                            acc_a[0:9, :], qsb[:, t, :], e[:, 0:512],
                            start=(t == 0), stop=(t == NCH - 1),
                        )
                        nc.tensor.matmul(
                            acc_b[0:9, :], qsb[:, t, :], e[:, 512:SLAB],
                            start=(t == 0), stop=(t == NCH - 1),
                        )
                    nc.vector.tensor_copy(s_sb[:, 0:512], acc_a[0:9, :])
                    nc.vector.tensor_copy(s_sb[:, 512:SLAB], acc_b[0:9, :])

            # ---- epilogue: moments + MLP in [d x i] layout
            with (
                tc.tile_pool(name="epi", bufs=1) as epi,
                tc.tile_pool(name="epips", bufs=1, space="PSUM") as epips,
            ):
                    s8 = epi.tile([1, SLAB], F32)
                    nc.sync.dma_start(out=s8, in_=s_sb[8:9, :])
                    sinv = epi.tile([1, SLAB], F32)
                    nc.vector.reciprocal(sinv, s8)
                    # broadcast 1/rowsum to 8 partitions
                    sinvb = epi.tile([8, SLAB], F32)
                    nc.gpsimd.partition_broadcast(sinvb, sinv)
                    w = epi.tile([8, SLAB], F32)
                    nc.vector.tensor_mul(w, s_sb[0:8, :], sinvb)
                    # realign alpha@q2 rows (4..8) onto partitions 0..4
                    wq2 = epi.tile([4, SLAB], F32)
                    nc.sync.dma_start(out=wq2, in_=w[4:8, :])
                    mu = epi.tile([4, SLAB], F32)
                    nc.vector.tensor_sub(mu, qslab, w[0:4, :])
                    sq = epi.tile([4, SLAB], F32)
                    nc.vector.tensor_mul(sq, w[0:4, :], w[0:4, :])
                    var = epi.tile([4, SLAB], F32)
                    nc.vector.scalar_tensor_tensor(
                        var, wq2, 1e-6, sq, op0=OP.add, op1=OP.subtract
                    )
                    sig = epi.tile([4, SLAB], F32)
                    nc.scalar.activation(sig, var, AF.Sqrt)
                    ft = epi.tile([8, SLAB], F32)
                    nc.sync.dma_start(out=ft[0:4, :], in_=mu)
                    nc.sync.dma_start(out=ft[4:8, :], in_=sig)
                    nc.sync.dma_start(out=fi_out[:, :], in_=ft)

                    h_ps = epips.tile([16, 1024], F32)
                    nc.tensor.matmul(
                        h_ps[:, 0:512], w1t, ft[:, 0:512], start=True, stop=True
                    )
                    nc.tensor.matmul(
                        h_ps[:, 512:SLAB], w1t, ft[:, 512:SLAB],
                        start=True, stop=True,
                    )
                    hsb = epi.tile([16, SLAB], F32)
                    nc.scalar.activation(hsb, h_ps[:, 0:SLAB], AF.Relu, bias=b1c)

                    dv_ps = epips.tile([2, 1024], F32)
                    nc.tensor.matmul(
                        dv_ps[:, 0:512], wct, hsb[:, 0:512], start=True, stop=True
                    )
                    nc.tensor.matmul(
                        dv_ps[:, 512:SLAB], wct, hsb[:, 512:SLAB],
                        start=True, stop=True,
                    )
                    dvt = epi.tile([2, SLAB], F32)
                    nc.scalar.activation(
                        dvt, dv_ps[:, 0:SLAB], AF.Identity, bias=bcc
                    )
                    nc.sync.dma_start(out=dv_out[:, :], in_=dvt)

                    u_ps = epips.tile([2, 1024], F32)
                    nc.tensor.matmul(
                        u_ps[:, 0:512], wut, hsb[:, 0:512], start=True, stop=True
                    )
                    nc.tensor.matmul(
                        u_ps[:, 512:SLAB], wut, hsb[:, 512:SLAB],
                        start=True, stop=True,
                    )
                    # softplus(u) = ln(1 + exp(u)); Exp and Ln share a table set
                    eu = epi.tile([2, SLAB], F32)
                    nc.scalar.activation(eu, u_ps[:, 0:SLAB], AF.Exp, bias=buc)
                    sp = epi.tile([2, SLAB], F32)
                    nc.scalar.activation(sp, eu, AF.Ln, bias=1.0)
                    s2t = epi.tile([2, SLAB], F32)
                    nc.vector.tensor_scalar_add(s2t, sp, 0.01)
                    nc.sync.dma_start(out=s2_out[:, :], in_=s2t)

    nc.finalize()
    return nc


def _host_prep(states, log_tau):
    f = np.float32
    s = np.asarray(states, dtype=f)
    pos = ((s[:, 0:2] + s[:, 2:4]) * f(0.5)).astype(f)
    vel = ((s[:, 4:6] + s[:, 6:8]) * f(0.5)).astype(f)
    x, y = pos[:, 0], pos[:, 1]
    r = ((x * x).astype(f) + (y * y).astype(f)).astype(f)
    ones = np.ones(N, f)
    A = np.stack([f(-2) * x, f(-2) * y, ones, r]).astype(f)      # [4, N]
    B = np.stack([x, y, r, ones]).astype(f)                      # [4, N]
    q = np.concatenate([pos, vel], axis=1).astype(f)             # [N, 4]
    Q = np.concatenate([q, (q * q).astype(f), ones[:, None]], axis=1)  # [N, 9]
    tau = float(np.exp(np.asarray(log_tau, dtype=f)))

    mask = (1.0 - np.eye(128)).astype(f)

    in_maps = []
    for k in range(NCORES):
        # rotate j-chunks so this core's 6 diagonal chunks come first
        rot = [(DIAG * k + t) % NCH for t in range(NCH)]
        cidx = np.concatenate(
            [np.arange(c * 128, (c + 1) * 128) for c in rot]
        )
        in_maps.append(
            {
                "Arep": np.ascontiguousarray(A[:, cidx]),
                "Brep": np.ascontiguousarray(B[:, SLAB * k : SLAB * (k + 1)]),
                "Qrows": np.ascontiguousarray(Q[cidx, :]),
                "dmask": mask,
                "qT": np.ascontiguousarray(q.T[:, SLAB * k : SLAB * (k + 1)]),
            }
        )
    return in_maps, tau


def _mlp_consts(W1, b1, Wc, bc, Wu, bu):
    f = np.float32
    return {
        "W1T": np.ascontiguousarray(np.asarray(W1, f).T),      # [8, 16]
        "b1c": np.asarray(b1, f).reshape(16, 1),
        "WcT": np.ascontiguousarray(np.asarray(Wc, f).T),      # [16, 2]
        "bcc": np.asarray(bc, f).reshape(2, 1),
        "WuT": np.ascontiguousarray(np.asarray(Wu, f).T),      # [16, 2]
        "buc": np.asarray(bu, f).reshape(2, 1),
    }


def kernel(states, log_tau, W1, b1, Wc, bc, Wu, bu, _trace=False):
    in_maps, tau = _host_prep(states, log_tau)
    consts = _mlp_consts(W1, b1, Wc, bc, Wu, bu)
    for m in in_maps:
        m.update(consts)

    key = round(tau, 12)
    if key not in _CACHE:
        _CACHE[key] = _build(tau)
    nc = _CACHE[key]

    res = run_bass_kernel_spmd(
        nc, in_maps, core_ids=list(range(NCORES)), trace=_trace
    )
    outs = res.results

    dv = np.concatenate([o["dvT"].T for o in outs], axis=0)
    s2 = np.concatenate([o["s2T"].T for o in outs], axis=0)
    fi = np.concatenate([o["fiT"].T for o in outs], axis=0)
    if _trace:
        kernel.last_exec_time_ns = res.exec_time_ns
        kernel.last_trace = res.instructions_and_trace
    return (
        np.ascontiguousarray(dv, np.float32),
        np.ascontiguousarray(s2, np.float32),
        np.ascontiguousarray(fi, np.float32),
    )
